# revision 1
# baseline (speedup 1.0000x reference)
"""Trainium2 Bass kernel for CrossModalFusion (B=4, C=64, H=W=64, N=4096).

Reference computation (per sample b, with x reshaped to [C, N]):
    q = wq @ xo + bq          [8, N]
    k = wk @ xs + bk          [8, N]
    v = wv @ xs + bv          [64, N]
    S[n, m]  = q[:, n] . k[:, m]
    attn     = softmax_m(S)
    out      = gamma * (v @ attn^T) + x_opt

Sharding: 8 cores = 4 batch samples x 2 halves of the query (n) axis.
Each core computes output rows [64, 2048] for its (sample, n-half); no
cross-core communication is needed.

Per-core dataflow:
  - biases are folded into augmented weights on the host (ones-row trick),
    so q/k/v come out of single matmuls against xs_aug/xo_aug ([65, *]
    tiles whose last row is 1.0).
  - scores are computed TRANSPOSED (S^T[m, n]) so that the exp'd scores can
    feed the attention*V matmul directly as the moving operand.  v^T gets an
    extra ones column, so the AV matmul's output row 64 accumulates
    sum_m exp(S[n, m]) — the softmax denominator comes out of the same
    accumulation for free.  No max-subtraction is needed: scores are O(3).
  - q/k are replicated at partition offsets 0 and 64 so the rank-8 S^T
    matmuls run two-at-a-time in distinct PE row groups.
  - per n-tile of 512, accumulate over all 32 m-blocks, then normalize by
    1/denominator, scale by gamma, add the x_opt residual and DMA out.
"""

import os
import sys

import numpy as np

for _p in ("/opt/trn_rl_repo", "/root/.axon_site/_ro/trn_rl_repo"):
    if os.path.isdir(_p) and _p not in sys.path:
        sys.path.insert(0, _p)

import concourse.bass as bass
import concourse.mybir as mybir
import concourse.tile as tile
from concourse import bacc
from concourse.bass_utils import run_bass_kernel_spmd

F32 = mybir.dt.float32
F32R = mybir.dt.float32r  # fp32 bits, fast PE matmul mode (~1.5e-4 rel err)
AF = mybir.ActivationFunctionType

B, C, HH, WW = 4, 64, 64, 64
N = HH * WW            # 4096 key/query positions
D = 8                  # q/k channel count
CA = C + 1             # augmented channel dim (ones row / denominator row)
NCORES = 8
NL = N // 2            # query rows per core
NT = 512               # n-tile (PSUM bank width in fp32)
MB = 128               # m-block (PE partition width)
N_NT = NL // NT        # 4 n-tiles per core
N_MB = N // MB         # 32 m-blocks
WAVE = 2               # m-blocks exp'd per ACT instruction


def build_program(repeat: int = 1) -> bass.Bass:
    # Bacc (not raw Bass): its compile() pass splits multi-semaphore waits
    # and moves matmul waits onto LDWEIGHTS, which this walrus build requires.
    # repeat>1 duplicates the whole body (benchmarking: wall-clock slope over
    # repeat isolates per-iteration kernel time from fixed dispatch overhead).
    nc = bacc.Bacc("TRN2", target_bir_lowering=False, num_devices=NCORES)
    # xo/xs arrive host-augmented with a trailing ones row ([65, *]) so PE
    # matmuls only wait on DMA producers (PE LDWEIGHTS allows max 2 sync
    # waits; an extra on-chip memset producer pushed it to 3).
    xo_d = nc.declare_dram_parameter("xo_aug", [CA, NL], F32R, isOutput=False)
    xs_d = nc.declare_dram_parameter("xs_aug", [CA, N], F32R, isOutput=False)
    wq_d = nc.declare_dram_parameter("wq_aug", [CA, D], F32R, isOutput=False)
    wk_d = nc.declare_dram_parameter("wk_aug", [CA, D], F32R, isOutput=False)
    wv_d = nc.declare_dram_parameter("wv_aug", [CA, CA], F32R, isOutput=False)
    g_d = nc.declare_dram_parameter("gamma", [1, 1], F32, isOutput=False)
    out_d = nc.declare_dram_parameter("out", [C, NL], F32, isOutput=True)

    with tile.TileContext(nc) as tc:
      for _rep in range(repeat):
        with tc.tile_pool(name="const", bufs=1) as cp:
            wq_sb = cp.tile([CA, D], F32R)
            nc.sync.dma_start(wq_sb[:], wq_d[:])
            wk_sb = cp.tile([CA, D], F32R)
            nc.sync.dma_start(wk_sb[:], wk_d[:])
            wv_sb = cp.tile([CA, CA], F32R)
            nc.sync.dma_start(wv_sb[:], wv_d[:])
            g_sb = cp.tile([1, 1], F32)
            nc.sync.dma_start(g_sb[:], g_d[:])
            ones_sb = cp.tile([1, C], F32)
            nc.vector.memset(ones_sb[:], 1.0)

            xs_aug = cp.tile([CA, N], F32R)
            for j in range(4):
                nc.sync.dma_start(
                    xs_aug[:, j * 1024 : (j + 1) * 1024],
                    xs_d[:, j * 1024 : (j + 1) * 1024],
                )

            xo_aug = cp.tile([CA, NL], F32R)
            for j in range(2):
                nc.sync.dma_start(
                    xo_aug[:, j * 1024 : (j + 1) * 1024],
                    xo_d[:, j * 1024 : (j + 1) * 1024],
                )

            # q/k at partition offsets 0 and 64 (PE row groups for the
            # concurrent rank-8 score matmuls); vT augmented with ones col.
            q_rep = cp.tile([64 + D, NL], F32R)
            k_rep = cp.tile([64 + D, N], F32R)
            vT = cp.tile([MB, N_MB * CA], F32R)

            with tc.tile_pool(name="pre_ps", bufs=2, space="PSUM") as pp:
                for j in range(N_NT):
                    qp = pp.tile([D, NT], F32, tag="qk_ps")
                    nc.tensor.matmul(
                        qp[:], wq_sb[:], xo_aug[:, j * NT : (j + 1) * NT],
                        start=True, stop=True,
                    )
                    nc.vector.tensor_copy(q_rep[0:D, j * NT : (j + 1) * NT], qp[:])
                    nc.sync.dma_start(
                        q_rep[64 : 64 + D, j * NT : (j + 1) * NT],
                        q_rep[0:D, j * NT : (j + 1) * NT],
                    )
                for j in range(N // NT):
                    kp = pp.tile([D, NT], F32, tag="qk_ps")
                    nc.tensor.matmul(
                        kp[:], wk_sb[:], xs_aug[:, j * NT : (j + 1) * NT],
                        start=True, stop=True,
                    )
                    nc.vector.tensor_copy(k_rep[0:D, j * NT : (j + 1) * NT], kp[:])
                    nc.sync.dma_start(
                        k_rep[64 : 64 + D, j * NT : (j + 1) * NT],
                        k_rep[0:D, j * NT : (j + 1) * NT],
                    )
                # v^T blocks: [128, 65] = xs_aug-block^T @ wv_aug.  Column 64
                # is all-ones (denominator column) since xs_aug row 64 is 1.
                for mb in range(N_MB):
                    vp = pp.tile([MB, CA], F32, tag="vt_ps")
                    # plain fp32: fp32r rejects the odd moving dim (65)
                    nc.tensor.matmul(
                        vp[:],
                        xs_aug[:, mb * MB : (mb + 1) * MB].bitcast(F32),
                        wv_sb[:].bitcast(F32),
                        start=True, stop=True,
                    )
                    nc.vector.tensor_copy(vT[:, mb * CA : (mb + 1) * CA], vp[:])

            with (
                tc.tile_pool(name="st_ps", bufs=2, space="PSUM") as st_pool,
                tc.tile_pool(name="av_ps", bufs=2, space="PSUM") as av_pool,
                tc.tile_pool(name="bc_ps", bufs=2, space="PSUM") as bc_pool,
                tc.tile_pool(name="e_sb", bufs=4) as e_pool,
                tc.tile_pool(name="o_sb", bufs=3) as o_pool,
                tc.tile_pool(name="sm_sb", bufs=3) as sm_pool,
            ):
                for nt in range(N_NT):
                    n0, n1 = nt * NT, (nt + 1) * NT
                    av = av_pool.tile([CA, NT], F32)

                    def emit_av(e_t, w, av=av):
                        for j in range(WAVE):
                            mb = WAVE * w + j
                            nc.tensor.matmul(
                                av[:],
                                vT[:, mb * CA : (mb + 1) * CA],
                                e_t[:, j * NT : (j + 1) * NT],
                                start=(mb == 0),
                                stop=(mb == N_MB - 1),
                            )

                    # S^T matmuls + exp, with the AV accumulation lagging one
                    # wave so the PE never stalls waiting on the current exp.
                    pend = None
                    for w in range(N_MB // WAVE):
                        st = st_pool.tile([MB, WAVE * NT], F32)
                        for j in range(WAVE):
                            mb = WAVE * w + j
                            rg = 64 * j
                            nc.tensor.matmul(
                                st[:, j * NT : (j + 1) * NT],
                                k_rep[rg : rg + D, mb * MB : (mb + 1) * MB],
                                q_rep[rg : rg + D, n0:n1],
                                start=True,
                                stop=True,
                            )
                        e_t = e_pool.tile([MB, WAVE * NT], F32R)
                        nc.scalar.activation(e_t[:], st[:], AF.Exp)
                        if pend is not None:
                            emit_av(*pend)
                        pend = (e_t, w)
                    emit_av(*pend)

                    # normalize: out = gamma/denom * unnorm + x_opt
                    recip = sm_pool.tile([1, NT], F32, tag="recip")
                    nc.vector.reciprocal(recip[:], av[C:CA, :])
                    sr = sm_pool.tile([1, NT], F32, tag="sr")
                    nc.vector.tensor_scalar_mul(sr[:], recip[:], g_sb[0:1, 0:1])
                    bc = bc_pool.tile([C, NT], F32)
                    nc.tensor.matmul(bc[:], ones_sb[:], sr[:], start=True, stop=True)
                    bcs = o_pool.tile([C, NT], F32, tag="bcs")
                    nc.vector.tensor_copy(bcs[:], bc[:])
                    om = o_pool.tile([C, NT], F32, tag="om")
                    nc.vector.tensor_mul(om[:], av[0:C, :], bcs[:])
                    o = o_pool.tile([C, NT], F32, tag="o")
                    nc.vector.tensor_add(o[:], om[:], xo_aug[0:C, n0:n1].bitcast(F32))
                    nc.sync.dma_start(out_d[:, n0:n1], o[:])
    nc.compile()
    return nc


_NC = None


def _get_nc() -> bass.Bass:
    global _NC
    if _NC is None:
        _NC = build_program()
    return _NC


def make_in_maps(x_opt, x_sar, wq, bq, wk, bk, wv, bv, gamma):
    f = np.float32
    x_opt = np.asarray(x_opt, f).reshape(B, C, N)
    x_sar = np.asarray(x_sar, f).reshape(B, C, N)
    wq_aug = np.ascontiguousarray(
        np.concatenate([np.asarray(wq, f).T, np.asarray(bq, f)[None, :]], axis=0)
    )
    wk_aug = np.ascontiguousarray(
        np.concatenate([np.asarray(wk, f).T, np.asarray(bk, f)[None, :]], axis=0)
    )
    wv_aug = np.zeros((CA, CA), f)
    wv_aug[:C, :C] = np.asarray(wv, f).T
    wv_aug[C, :C] = np.asarray(bv, f)
    wv_aug[C, C] = 1.0
    g = np.asarray(gamma, f).reshape(1, 1)
    ones_n = np.ones((1, N), f)
    maps = []
    for core in range(NCORES):
        b, h = divmod(core, 2)
        xo_aug = np.concatenate(
            [x_opt[b, :, h * NL : (h + 1) * NL], ones_n[:, :NL]], axis=0
        )
        xs_aug = np.concatenate([x_sar[b], ones_n], axis=0)
        maps.append(
            {
                "xo_aug": np.ascontiguousarray(xo_aug),
                "xs_aug": np.ascontiguousarray(xs_aug),
                "wq_aug": wq_aug,
                "wk_aug": wk_aug,
                "wv_aug": wv_aug,
                "gamma": g,
            }
        )
    return maps


def assemble_out(results) -> np.ndarray:
    out = np.empty((B, C, N), np.float32)
    for core in range(NCORES):
        b, h = divmod(core, 2)
        out[b, :, h * NL : (h + 1) * NL] = results[core]["out"]
    return out.reshape(B, C, HH, WW)


def kernel(**inputs) -> np.ndarray:
    nc = _get_nc()
    maps = make_in_maps(**inputs)
    res = run_bass_kernel_spmd(nc, maps, list(range(NCORES)))
    return assemble_out(res.results)



# revision 5
# speedup vs baseline: 1.0018x; 1.0018x over previous
"""Trainium2 Bass kernel for CrossModalFusion (B=4, C=64, H=W=64, N=4096).

Reference computation (per sample b, with x reshaped to [C, N]):
    q = wq @ xo + bq          [8, N]
    k = wk @ xs + bk          [8, N]
    v = wv @ xs + bv          [64, N]
    S[n, m]  = q[:, n] . k[:, m]
    attn     = softmax_m(S)
    out      = gamma * (v @ attn^T) + x_opt

Sharding: 8 cores = 4 batch samples x 2 halves of the query (n) axis.
Each core computes output rows [64, 2048] for its (sample, n-half); no
cross-core communication is needed.

Per-core dataflow (all matmuls in bf16 — fp32/fp32r PE modes run at the
cold 1.2 GHz clock and never engage the HAM warm-up; bf16 streams one
column per 2.4 GHz cycle once warm, ~3.5x faster):
  - biases are folded into augmented weights on the host (ones-row trick);
    gamma is folded into wv/bv on the host, so the attention output comes
    out pre-scaled and the softmax denominator column stays unscaled.
  - scores are computed TRANSPOSED (S^T[m, n]) so the exp'd scores feed
    the attention*V matmul directly as the moving operand.  v^T gets an
    extra ones column, so the AV matmul's row 64 accumulates
    sum_m exp(S[n, m]) — the softmax denominator for free.
    No max-subtraction: scores are O(3).
  - q/k are replicated at partition offsets 0 and 64 so the rank-8 S^T
    matmuls run two-at-a-time in distinct PE row groups.
  - per n-tile of 512, accumulate over all 32 m-blocks, then normalize by
    1/denominator, add the fp32 x_opt residual and DMA out.
"""

import os
import sys

import numpy as np

for _p in ("/opt/trn_rl_repo", "/root/.axon_site/_ro/trn_rl_repo"):
    if os.path.isdir(_p) and _p not in sys.path:
        sys.path.insert(0, _p)

import concourse.bass as bass
import concourse.mybir as mybir
import concourse.tile as tile
from concourse import bacc
from concourse.bass_utils import run_bass_kernel_spmd

F32 = mybir.dt.float32
F32R = mybir.dt.float32r
BF16 = mybir.dt.bfloat16
AF = mybir.ActivationFunctionType

B, C, HH, WW = 4, 64, 64, 64
N = HH * WW            # 4096 key/query positions
D = 8                  # q/k channel count
CA = C + 1             # augmented channel dim (ones row / denominator row)
NCORES = 8
NL = N // 2            # query rows per core
NT = 512               # n-tile (PSUM bank width in fp32)
MB = 128               # m-block (PE partition width)
N_NT = NL // NT        # 4 n-tiles per core
N_MB = N // MB         # 32 m-blocks
WAVE = 2               # m-blocks exp'd per ACT instruction


def build_program(repeat: int = 1) -> bass.Bass:
    nc = bacc.Bacc("TRN2", target_bir_lowering=False, num_devices=NCORES)
    # bf16 inputs are converted host-side; xores is the fp32 residual.
    xo_d = nc.declare_dram_parameter("xo_bf", [CA, NL], BF16, isOutput=False)
    xs_d = nc.declare_dram_parameter("xs_bf", [CA, N], BF16, isOutput=False)
    xr_d = nc.declare_dram_parameter("xores", [C, NL], F32, isOutput=False)
    wq_d = nc.declare_dram_parameter("wq_bf", [CA, D], BF16, isOutput=False)
    wk_d = nc.declare_dram_parameter("wk_bf", [CA, D], BF16, isOutput=False)
    wv_d = nc.declare_dram_parameter("wv_bf", [CA, CA], BF16, isOutput=False)
    out_d = nc.declare_dram_parameter("out", [C, NL], F32, isOutput=True)

    with tile.TileContext(nc) as tc:
      for _rep in range(repeat):
        with tc.tile_pool(name="const", bufs=1) as cp:
            wq_sb = cp.tile([CA, D], BF16)
            nc.sync.dma_start(wq_sb[:], wq_d[:])
            wk_sb = cp.tile([CA, D], BF16)
            nc.sync.dma_start(wk_sb[:], wk_d[:])
            wv_sb = cp.tile([CA, CA], BF16)
            nc.sync.dma_start(wv_sb[:], wv_d[:])
            ones_sb = cp.tile([1, C], BF16)
            nc.vector.memset(ones_sb[:], 1.0)

            xs_sb = cp.tile([CA, N], BF16)
            for j in range(4):
                nc.sync.dma_start(
                    xs_sb[:, j * 1024 : (j + 1) * 1024],
                    xs_d[:, j * 1024 : (j + 1) * 1024],
                )

            xo_sb = cp.tile([CA, NL], BF16)
            for j in range(2):
                nc.sync.dma_start(
                    xo_sb[:, j * 1024 : (j + 1) * 1024],
                    xo_d[:, j * 1024 : (j + 1) * 1024],
                )

            xr_sb = cp.tile([C, NL], F32)
            for j in range(2):
                nc.sync.dma_start(
                    xr_sb[:, j * 1024 : (j + 1) * 1024],
                    xr_d[:, j * 1024 : (j + 1) * 1024],
                )

            # q/k at partition offsets 0 and 64 (PE row groups for the
            # concurrent rank-8 score matmuls); vT augmented with ones col.
            q_rep = cp.tile([64 + D, NL], BF16)
            k_rep = cp.tile([64 + D, N], BF16)
            vT = cp.tile([MB, N_MB * CA], BF16)

            with tc.tile_pool(name="pre_ps", bufs=2, space="PSUM") as pp:
                for j in range(N_NT):
                    qp = pp.tile([D, NT], F32, tag="qk_ps")
                    nc.tensor.matmul(
                        qp[:], wq_sb[:], xo_sb[:, j * NT : (j + 1) * NT],
                        start=True, stop=True,
                    )
                    nc.vector.tensor_copy(q_rep[0:D, j * NT : (j + 1) * NT], qp[:])
                    nc.sync.dma_start(
                        q_rep[64 : 64 + D, j * NT : (j + 1) * NT],
                        q_rep[0:D, j * NT : (j + 1) * NT],
                    )
                for j in range(N // NT):
                    kp = pp.tile([D, NT], F32, tag="qk_ps")
                    nc.tensor.matmul(
                        kp[:], wk_sb[:], xs_sb[:, j * NT : (j + 1) * NT],
                        start=True, stop=True,
                    )
                    nc.vector.tensor_copy(k_rep[0:D, j * NT : (j + 1) * NT], kp[:])
                    nc.sync.dma_start(
                        k_rep[64 : 64 + D, j * NT : (j + 1) * NT],
                        k_rep[0:D, j * NT : (j + 1) * NT],
                    )
                # v^T blocks: [128, 65] = xs-block^T @ wv_aug.  Column 64
                # is all-ones (denominator column) since xs row 64 is 1.
                for mb in range(N_MB):
                    vp = pp.tile([MB, CA], F32, tag="vt_ps")
                    nc.tensor.matmul(
                        vp[:],
                        xs_sb[:, mb * MB : (mb + 1) * MB],
                        wv_sb[:],
                        start=True, stop=True,
                    )
                    nc.vector.tensor_copy(vT[:, mb * CA : (mb + 1) * CA], vp[:])

            with (
                tc.tile_pool(name="st_ps", bufs=2, space="PSUM") as st_pool,
                tc.tile_pool(name="av_ps", bufs=2, space="PSUM") as av_pool,
                tc.tile_pool(name="bc_ps", bufs=2, space="PSUM") as bc_pool,
                tc.tile_pool(name="e_sb", bufs=4) as e_pool,
                tc.tile_pool(name="o_sb", bufs=3) as o_pool,
                tc.tile_pool(name="sm_sb", bufs=3) as sm_pool,
            ):
                for nt in range(N_NT):
                    n0, n1 = nt * NT, (nt + 1) * NT
                    av = av_pool.tile([CA, NT], F32)

                    def emit_av(e_t, w, av=av):
                        for j in range(WAVE):
                            mb = WAVE * w + j
                            nc.tensor.matmul(
                                av[:],
                                vT[:, mb * CA : (mb + 1) * CA],
                                e_t[:, j * NT : (j + 1) * NT],
                                start=(mb == 0),
                                stop=(mb == N_MB - 1),
                            )

                    # S^T matmuls + exp, with the AV accumulation lagging one
                    # wave so the PE never stalls waiting on the current exp.
                    pend = None
                    for w in range(N_MB // WAVE):
                        st = st_pool.tile([MB, WAVE * NT], F32)
                        for j in range(WAVE):
                            mb = WAVE * w + j
                            rg = 64 * j
                            nc.tensor.matmul(
                                st[:, j * NT : (j + 1) * NT],
                                k_rep[rg : rg + D, mb * MB : (mb + 1) * MB],
                                q_rep[rg : rg + D, n0:n1],
                                start=True,
                                stop=True,
                            )
                        e_t = e_pool.tile([MB, WAVE * NT], BF16)
                        nc.scalar.activation(e_t[:], st[:], AF.Exp)
                        if pend is not None:
                            emit_av(*pend)
                        pend = (e_t, w)
                    emit_av(*pend)

                    # normalize: out = unnorm/denom + x_opt  (gamma folded
                    # into wv on the host, so unnorm is pre-scaled)
                    recip = sm_pool.tile([1, NT], F32, tag="recip")
                    nc.vector.reciprocal(recip[:], av[C:CA, :])
                    recip_bf = sm_pool.tile([1, NT], BF16, tag="recip_bf")
                    nc.vector.tensor_copy(recip_bf[:], recip[:])
                    bc = bc_pool.tile([C, NT], F32)
                    nc.tensor.matmul(
                        bc[:], ones_sb[:], recip_bf[:],
                        start=True, stop=True,
                    )
                    bcs = o_pool.tile([C, NT], F32, tag="bcs")
                    nc.vector.tensor_copy(bcs[:], bc[:])
                    om = o_pool.tile([C, NT], F32, tag="om")
                    nc.vector.tensor_mul(om[:], av[0:C, :], bcs[:])
                    o = o_pool.tile([C, NT], F32, tag="o")
                    nc.vector.tensor_add(o[:], om[:], xr_sb[:, n0:n1])
                    nc.sync.dma_start(out_d[:, n0:n1], o[:])
    nc.compile()
    return nc


_NC = None


def _get_nc() -> bass.Bass:
    global _NC
    if _NC is None:
        _NC = build_program()
    return _NC


def _to_bf16(a: np.ndarray) -> np.ndarray:
    """Round-to-nearest-even fp32 -> bf16 (ml_dtypes view)."""
    import ml_dtypes

    u = np.ascontiguousarray(a, np.float32).view(np.uint32)
    rounded = ((u + 0x7FFF + ((u >> 16) & 1)) >> 16).astype(np.uint16)
    return rounded.view(ml_dtypes.bfloat16)


def make_in_maps(x_opt, x_sar, wq, bq, wk, bk, wv, bv, gamma):
    f = np.float32
    x_opt = np.asarray(x_opt, f).reshape(B, C, N)
    x_sar = np.asarray(x_sar, f).reshape(B, C, N)
    g = float(np.asarray(gamma, f).reshape(()))
    wq_aug = np.concatenate([np.asarray(wq, f).T, np.asarray(bq, f)[None, :]], axis=0)
    wk_aug = np.concatenate([np.asarray(wk, f).T, np.asarray(bk, f)[None, :]], axis=0)
    # gamma folded into v (weights AND bias); denominator column stays 1.
    wv_aug = np.zeros((CA, CA), f)
    wv_aug[:C, :C] = np.asarray(wv, f).T * g
    wv_aug[C, :C] = np.asarray(bv, f) * g
    wv_aug[C, C] = 1.0
    wq_bf = _to_bf16(wq_aug)
    wk_bf = _to_bf16(wk_aug)
    wv_bf = _to_bf16(wv_aug)
    ones_n = np.ones((1, N), f)
    maps = []
    for core in range(NCORES):
        b, h = divmod(core, 2)
        xo_aug = np.concatenate(
            [x_opt[b, :, h * NL : (h + 1) * NL], ones_n[:, :NL]], axis=0
        )
        xs_aug = np.concatenate([x_sar[b], ones_n], axis=0)
        maps.append(
            {
                "xo_bf": _to_bf16(xo_aug),
                "xs_bf": _to_bf16(xs_aug),
                "xores": np.ascontiguousarray(x_opt[b, :, h * NL : (h + 1) * NL]),
                "wq_bf": wq_bf,
                "wk_bf": wk_bf,
                "wv_bf": wv_bf,
            }
        )
    return maps


def assemble_out(results) -> np.ndarray:
    out = np.empty((B, C, N), np.float32)
    for core in range(NCORES):
        b, h = divmod(core, 2)
        out[b, :, h * NL : (h + 1) * NL] = results[core]["out"]
    return out.reshape(B, C, HH, WW)


def kernel(**inputs) -> np.ndarray:
    nc = _get_nc()
    maps = make_in_maps(**inputs)
    res = run_bass_kernel_spmd(nc, maps, list(range(NCORES)))
    return assemble_out(res.results)


# revision 7
# speedup vs baseline: 1.1326x; 1.1306x over previous
"""Trainium2 Bass kernel for CrossModalFusion (B=4, C=64, H=W=64, N=4096).

Reference computation (per sample b, with x reshaped to [C, N]):
    q = wq @ xo + bq          [8, N]
    k = wk @ xs + bk          [8, N]
    v = wv @ xs + bv          [64, N]
    S[n, m]  = q[:, n] . k[:, m]
    attn     = softmax_m(S)
    out      = gamma * (v @ attn^T) + x_opt

Sharding: 8 cores = 4 batch samples x 2 halves of the query (n) axis.
Each core computes output rows [64, 2048] for its (sample, n-half); no
cross-core communication is needed.

Per-core dataflow (matmuls in bf16 / f32r — the PE in this environment never
leaves the 1.2 GHz throttled clock, so concurrency via PE array tiling is the
main lever):
  - biases are folded into augmented weights on the host (ones-row trick);
    gamma is folded into wv/bv on the host, so the attention output comes out
    pre-scaled and the softmax denominator column stays unscaled.
  - scores are computed TRANSPOSED (S^T[m, n]) so the exp'd scores feed the
    attention*V matmuls directly as the moving operand.  v^T gets an extra
    ones column, so the AV matmuls' row 64 accumulate sum_m exp(S[n, m]) —
    the softmax denominator for free.  No max-subtraction: scores are O(3).
  - q/k are replicated at partition offsets 0/32/64/96 so four rank-8 S^T
    matmuls run concurrently in the four 32-row PE groups.
  - AV matmuls are split into rows 0-63 / 64-127 (two concurrent 64-row PE
    groups) accumulating into separate PSUM tiles avA/avB, summed at
    normalize time.
  - q/k/vT prep is interleaved just-in-time into n-tile 0's wave loop so the
    exp pipeline starts as soon as the first score block exists.
  - per n-tile of 512: accumulate over all 32 m-blocks, normalize by
    1/denominator, add the fp32 x_opt residual, DMA out.
"""

import os
import sys

import numpy as np

for _p in ("/opt/trn_rl_repo", "/root/.axon_site/_ro/trn_rl_repo"):
    if os.path.isdir(_p) and _p not in sys.path:
        sys.path.insert(0, _p)

import concourse.bass as bass
import concourse.mybir as mybir
import concourse.tile as tile
from concourse import bacc
from concourse.bass_utils import run_bass_kernel_spmd

F32 = mybir.dt.float32
F32R = mybir.dt.float32r
BF16 = mybir.dt.bfloat16
AF = mybir.ActivationFunctionType

B, C, HH, WW = 4, 64, 64, 64
N = HH * WW            # 4096 key/query positions
D = 8                  # q/k channel count
CA = C + 1             # augmented channel dim (ones row / denominator row)
NCORES = 8
NL = N // 2            # query rows per core
NT = 512               # n-tile (PSUM bank width in fp32)
MB = 128               # m-block (PE partition width)
N_NT = NL // NT        # 4 n-tiles per core
N_MB = N // MB         # 32 m-blocks
E_DTYPE = F32R         # exp output / AV operand dtype


def build_program(repeat: int = 1) -> bass.Bass:
    nc = bacc.Bacc("TRN2", target_bir_lowering=False, num_devices=NCORES)
    xo_d = nc.declare_dram_parameter("xo_bf", [CA, NL], BF16, isOutput=False)
    xs_d = nc.declare_dram_parameter("xs_bf", [CA, N], BF16, isOutput=False)
    xr_d = nc.declare_dram_parameter("xores", [C, NL], F32, isOutput=False)
    wq_d = nc.declare_dram_parameter("wq_bf", [CA, D], BF16, isOutput=False)
    wk_d = nc.declare_dram_parameter("wk_bf", [CA, D], BF16, isOutput=False)
    wv_d = nc.declare_dram_parameter("wv_bf", [CA, CA], BF16, isOutput=False)
    out_d = nc.declare_dram_parameter("out", [C, NL], F32, isOutput=True)

    with tile.TileContext(nc) as tc:
      for _rep in range(repeat):
        with (
            tc.tile_pool(name="const", bufs=1) as cp,
            tc.tile_pool(name="st_ps", bufs=2, space="PSUM") as st_pool,
            tc.tile_pool(name="avA_ps", bufs=1, space="PSUM") as avA_pool,
            tc.tile_pool(name="avB_ps", bufs=1, space="PSUM") as avB_pool,
            tc.tile_pool(name="bc_ps", bufs=1, space="PSUM") as bc_pool,
            tc.tile_pool(name="pre_ps", bufs=1, space="PSUM") as pre_pool,
            tc.tile_pool(name="e_sb", bufs=4) as e_pool,
            tc.tile_pool(name="o_sb", bufs=2) as o_pool,
            tc.tile_pool(name="sm_sb", bufs=2) as sm_pool,
        ):
            wq_sb = cp.tile([CA, D], BF16)
            nc.sync.dma_start(wq_sb[:], wq_d[:])
            wk_sb = cp.tile([CA, D], BF16)
            nc.sync.dma_start(wk_sb[:], wk_d[:])
            wv_sb = cp.tile([CA, CA], BF16)
            nc.sync.dma_start(wv_sb[:], wv_d[:])
            ones_sb = cp.tile([1, C], BF16)
            nc.vector.memset(ones_sb[:], 1.0)

            xs_sb = cp.tile([CA, N], BF16)
            for j in range(8):
                nc.sync.dma_start(
                    xs_sb[:, j * NT : (j + 1) * NT], xs_d[:, j * NT : (j + 1) * NT]
                )
            xo_sb = cp.tile([CA, NL], BF16)
            for j in range(4):
                nc.sync.dma_start(
                    xo_sb[:, j * NT : (j + 1) * NT], xo_d[:, j * NT : (j + 1) * NT]
                )
            xr_sb = cp.tile([C, NL], F32)
            for j in range(2):
                nc.sync.dma_start(
                    xr_sb[:, j * 1024 : (j + 1) * 1024],
                    xr_d[:, j * 1024 : (j + 1) * 1024],
                )

            # q/k replicated at partition offsets 0/32/64/96 (score row
            # groups); vT blocks [128, 65] with trailing ones column.
            q_rep = cp.tile([96 + D, NL], BF16)
            k_rep = cp.tile([96 + D, N], BF16)
            vT = cp.tile([MB, N_MB * CA], E_DTYPE)

            def prep_k_chunk(c):
                kp = pre_pool.tile([D, NT], F32, tag="pre", name=f"kp{c}")
                nc.tensor.matmul(
                    kp[:], wk_sb[:], xs_sb[:, c * NT : (c + 1) * NT],
                    start=True, stop=True,
                )
                nc.vector.tensor_copy(k_rep[0:D, c * NT : (c + 1) * NT], kp[:])
                for rg in (32, 64, 96):
                    nc.sync.dma_start(
                        k_rep[rg : rg + D, c * NT : (c + 1) * NT],
                        k_rep[0:D, c * NT : (c + 1) * NT],
                    )

            def prep_q_chunk(c):
                qp = pre_pool.tile([D, NT], F32, tag="pre", name=f"qp{c}")
                nc.tensor.matmul(
                    qp[:], wq_sb[:], xo_sb[:, c * NT : (c + 1) * NT],
                    start=True, stop=True,
                )
                nc.vector.tensor_copy(q_rep[0:D, c * NT : (c + 1) * NT], qp[:])
                for rg in (32, 64, 96):
                    nc.sync.dma_start(
                        q_rep[rg : rg + D, c * NT : (c + 1) * NT],
                        q_rep[0:D, c * NT : (c + 1) * NT],
                    )

            def prep_vt_block(mb):
                vp = pre_pool.tile([MB, CA], F32, tag="pre", name=f"vp{mb}")
                nc.tensor.matmul(
                    vp[:], xs_sb[:, mb * MB : (mb + 1) * MB], wv_sb[:],
                    start=True, stop=True,
                )
                nc.vector.tensor_copy(vT[:, mb * CA : (mb + 1) * CA], vp[:])

            prep_k_chunk(0)
            prep_q_chunk(0)

            for nt in range(N_NT):
                n0, n1 = nt * NT, (nt + 1) * NT
                avA = avA_pool.tile([CA, NT], F32, tag="avA", name=f"avA{nt}")
                avB = avB_pool.tile([CA, NT], F32, tag="avB", name=f"avB{nt}")

                def emit_av(e_t, w, avA=avA, avB=avB):
                    for j in range(2):
                        mb = 2 * w + j
                        nc.tensor.matmul(
                            avA[:],
                            vT[0:64, mb * CA : (mb + 1) * CA],
                            e_t[0:64, j * NT : (j + 1) * NT],
                            start=(mb == 0), stop=(mb == N_MB - 1),
                        )
                        nc.tensor.matmul(
                            avB[:],
                            vT[64:MB, mb * CA : (mb + 1) * CA],
                            e_t[64:MB, j * NT : (j + 1) * NT],
                            start=(mb == 0), stop=(mb == N_MB - 1),
                        )

                pend = []
                for p in range(N_MB // 4):  # wave pairs: m-blocks 4p..4p+3
                    if nt == 0:
                        if p + 1 < 8:
                            prep_k_chunk(p + 1)
                        if p in (1, 3, 5) and p // 2 + 1 < N_NT:
                            prep_q_chunk(p // 2 + 1)
                    # score quad: 4 concurrent rank-8 matmuls in distinct
                    # 32-row PE groups
                    st0 = st_pool.tile([MB, 2 * NT], F32, tag="st", name=f"st{nt}_{p}a")
                    st1 = st_pool.tile([MB, 2 * NT], F32, tag="st", name=f"st{nt}_{p}b")
                    for j4 in range(4):
                        mb = 4 * p + j4
                        rg = 32 * j4
                        stt, col = (st0, j4 * NT) if j4 < 2 else (st1, (j4 - 2) * NT)
                        nc.tensor.matmul(
                            stt[:, col : col + NT],
                            k_rep[rg : rg + D, mb * MB : (mb + 1) * MB],
                            q_rep[rg : rg + D, n0:n1],
                            start=True, stop=True,
                            tile_position=(rg, 0),
                        )
                    if nt == 0:
                        for mb in range(4 * p, 4 * p + 4):
                            prep_vt_block(mb)
                    e0 = e_pool.tile([MB, 2 * NT], E_DTYPE, tag="e", name=f"e{nt}_{p}a")
                    nc.scalar.activation(e0[:], st0[:], AF.Exp)
                    e1 = e_pool.tile([MB, 2 * NT], E_DTYPE, tag="e", name=f"e{nt}_{p}b")
                    nc.scalar.activation(e1[:], st1[:], AF.Exp)
                    for args in pend:
                        emit_av(*args)
                    pend = [(e0, 2 * p), (e1, 2 * p + 1)]
                for args in pend:
                    emit_av(*args)

                # normalize: out = unnorm/denom + x_opt  (gamma pre-folded)
                avAs = o_pool.tile([CA, NT], F32, tag="avAs", name=f"avAs{nt}")
                nc.vector.tensor_copy(avAs[:], avA[:])
                avS = o_pool.tile([CA, NT], F32, tag="avS", name=f"avS{nt}")
                nc.vector.tensor_add(avS[:], avB[:], avAs[:])
                recip = sm_pool.tile([1, NT], F32, tag="recip", name=f"recip{nt}")
                nc.vector.reciprocal(recip[:], avS[C:CA, :])
                recip_bf = sm_pool.tile([1, NT], BF16, tag="recip_bf", name=f"rb{nt}")
                nc.vector.tensor_copy(recip_bf[:], recip[:])
                bc = bc_pool.tile([C, NT], F32, tag="bc", name=f"bc{nt}")
                nc.tensor.matmul(bc[:], ones_sb[:], recip_bf[:], start=True, stop=True)
                bcs = o_pool.tile([C, NT], F32, tag="bcs", name=f"bcs{nt}")
                nc.vector.tensor_copy(bcs[:], bc[:])
                om = o_pool.tile([C, NT], F32, tag="om", name=f"om{nt}")
                nc.vector.tensor_mul(om[:], avS[0:C, :], bcs[:])
                o = o_pool.tile([C, NT], F32, tag="o", name=f"o{nt}")
                nc.vector.tensor_add(o[:], om[:], xr_sb[:, n0:n1])
                nc.sync.dma_start(out_d[:, n0:n1], o[:])
    nc.compile()
    return nc


_NC = None


def _get_nc() -> bass.Bass:
    global _NC
    if _NC is None:
        _NC = build_program()
    return _NC


def _to_bf16(a: np.ndarray) -> np.ndarray:
    """Round-to-nearest-even fp32 -> bf16 (ml_dtypes view)."""
    import ml_dtypes

    u = np.ascontiguousarray(a, np.float32).view(np.uint32)
    rounded = ((u + 0x7FFF + ((u >> 16) & 1)) >> 16).astype(np.uint16)
    return rounded.view(ml_dtypes.bfloat16)


def make_in_maps(x_opt, x_sar, wq, bq, wk, bk, wv, bv, gamma):
    f = np.float32
    x_opt = np.asarray(x_opt, f).reshape(B, C, N)
    x_sar = np.asarray(x_sar, f).reshape(B, C, N)
    g = float(np.asarray(gamma, f).reshape(()))
    wq_aug = np.concatenate([np.asarray(wq, f).T, np.asarray(bq, f)[None, :]], axis=0)
    wk_aug = np.concatenate([np.asarray(wk, f).T, np.asarray(bk, f)[None, :]], axis=0)
    # gamma folded into v (weights AND bias); denominator column stays 1.
    wv_aug = np.zeros((CA, CA), f)
    wv_aug[:C, :C] = np.asarray(wv, f).T * g
    wv_aug[C, :C] = np.asarray(bv, f) * g
    wv_aug[C, C] = 1.0
    wq_bf = _to_bf16(wq_aug)
    wk_bf = _to_bf16(wk_aug)
    wv_bf = _to_bf16(wv_aug)
    ones_n = np.ones((1, N), f)
    maps = []
    for core in range(NCORES):
        b, h = divmod(core, 2)
        xo_aug = np.concatenate(
            [x_opt[b, :, h * NL : (h + 1) * NL], ones_n[:, :NL]], axis=0
        )
        xs_aug = np.concatenate([x_sar[b], ones_n], axis=0)
        maps.append(
            {
                "xo_bf": _to_bf16(xo_aug),
                "xs_bf": _to_bf16(xs_aug),
                "xores": np.ascontiguousarray(x_opt[b, :, h * NL : (h + 1) * NL]),
                "wq_bf": wq_bf,
                "wk_bf": wk_bf,
                "wv_bf": wv_bf,
            }
        )
    return maps


def assemble_out(results) -> np.ndarray:
    out = np.empty((B, C, N), np.float32)
    for core in range(NCORES):
        b, h = divmod(core, 2)
        out[b, :, h * NL : (h + 1) * NL] = results[core]["out"]
    return out.reshape(B, C, HH, WW)


def kernel(**inputs) -> np.ndarray:
    nc = _get_nc()
    maps = make_in_maps(**inputs)
    res = run_bass_kernel_spmd(nc, maps, list(range(NCORES)))
    return assemble_out(res.results)


# revision 8
# speedup vs baseline: 1.2295x; 1.0856x over previous
"""Trainium2 Bass kernel for CrossModalFusion (B=4, C=64, H=W=64, N=4096).

Reference computation (per sample b, with x reshaped to [C, N]):
    q = wq @ xo + bq          [8, N]
    k = wk @ xs + bk          [8, N]
    v = wv @ xs + bv          [64, N]
    S[n, m]  = q[:, n] . k[:, m]
    attn     = softmax_m(S)
    out      = gamma * (v @ attn^T) + x_opt

Sharding: 8 cores = 4 batch samples x 2 halves of the query (n) axis.
Each core computes output rows [64, 2048] for its (sample, n-half); no
cross-core communication is needed.

Per-core dataflow (matmuls in bf16 / f32r — the PE in this environment never
leaves the 1.2 GHz throttled clock, so concurrency via PE array tiling is the
main lever):
  - biases are folded into augmented weights on the host (ones-row trick);
    gamma is folded into wv/bv on the host, so the attention output comes out
    pre-scaled and the softmax denominator column stays unscaled.
  - scores are computed TRANSPOSED (S^T[m, n]) so the exp'd scores feed the
    attention*V matmuls directly as the moving operand.  v^T gets an extra
    ones column, so the AV matmuls' row 64 accumulate sum_m exp(S[n, m]) —
    the softmax denominator for free.  No max-subtraction: scores are O(3).
  - q/k are replicated at partition offsets 0/32/64/96 so four rank-8 S^T
    matmuls run concurrently in the four 32-row PE groups.
  - AV matmuls are split into rows 0-63 / 64-127 (two concurrent 64-row PE
    groups) accumulating into separate PSUM tiles avA/avB, summed at
    normalize time.
  - q/k/vT prep is interleaved just-in-time into n-tile 0's wave loop so the
    exp pipeline starts as soon as the first score block exists.
  - per n-tile of 512: accumulate over all 32 m-blocks, normalize by
    1/denominator, add the fp32 x_opt residual, DMA out.
"""

import os
import sys

import numpy as np

for _p in ("/opt/trn_rl_repo", "/root/.axon_site/_ro/trn_rl_repo"):
    if os.path.isdir(_p) and _p not in sys.path:
        sys.path.insert(0, _p)

import concourse.bass as bass
import concourse.mybir as mybir
import concourse.tile as tile
from concourse import bacc
from concourse.bass_utils import run_bass_kernel_spmd

F32 = mybir.dt.float32
F32R = mybir.dt.float32r
BF16 = mybir.dt.bfloat16
AF = mybir.ActivationFunctionType

B, C, HH, WW = 4, 64, 64, 64
N = HH * WW            # 4096 key/query positions
D = 8                  # q/k channel count
CA = C + 1             # augmented channel dim (ones row / denominator row)
NCORES = 8
NL = N // 2            # query rows per core
NT = 512               # n-tile (PSUM bank width in fp32)
MB = 128               # m-block (PE partition width)
N_NT = NL // NT        # 4 n-tiles per core
N_MB = N // MB         # 32 m-blocks
E_DTYPE = F32R         # exp output / AV operand dtype


def build_program(repeat: int = 1) -> bass.Bass:
    nc = bacc.Bacc("TRN2", target_bir_lowering=False, num_devices=NCORES)
    xo_d = nc.declare_dram_parameter("xo_bf", [CA, NL], BF16, isOutput=False)
    xs_d = nc.declare_dram_parameter("xs_bf", [CA, N], BF16, isOutput=False)
    xr_d = nc.declare_dram_parameter("xores", [C, NL], F32, isOutput=False)
    wq_d = nc.declare_dram_parameter("wq_bf", [CA, 96 + D], BF16, isOutput=False)
    wk_d = nc.declare_dram_parameter("wk_bf", [CA, 96 + D], BF16, isOutput=False)
    wv_d = nc.declare_dram_parameter("wv_bf", [CA, CA], BF16, isOutput=False)
    out_d = nc.declare_dram_parameter("out", [C, NL], F32, isOutput=True)

    with tile.TileContext(nc) as tc:
      for _rep in range(repeat):
        with (
            tc.tile_pool(name="const", bufs=1) as cp,
            tc.tile_pool(name="st_ps", bufs=2, space="PSUM") as st_pool,
            tc.tile_pool(name="avA_ps", bufs=1, space="PSUM") as avA_pool,
            tc.tile_pool(name="avB_ps", bufs=1, space="PSUM") as avB_pool,
            tc.tile_pool(name="bc_ps", bufs=1, space="PSUM") as bc_pool,
            tc.tile_pool(name="pre_ps", bufs=1, space="PSUM") as pre_pool,
            tc.tile_pool(name="e_sb", bufs=4) as e_pool,
            tc.tile_pool(name="o_sb", bufs=2) as o_pool,
            tc.tile_pool(name="sm_sb", bufs=2) as sm_pool,
        ):
            wk_sb = cp.tile([CA, 96 + D], BF16)
            nc.sync.dma_start(wk_sb[:], wk_d[:])
            wq_sb = cp.tile([CA, 96 + D], BF16)
            nc.sync.dma_start(wq_sb[:], wq_d[:])
            xs_sb = cp.tile([CA, N], BF16)
            xo_sb = cp.tile([CA, NL], BF16)
            nc.sync.dma_start(xs_sb[:, 0:1024], xs_d[:, 0:1024])
            nc.sync.dma_start(xo_sb[:, 0:1024], xo_d[:, 0:1024])
            wv_sb = cp.tile([CA, CA], BF16)
            nc.sync.dma_start(wv_sb[:], wv_d[:])
            ones_sb = cp.tile([1, C], BF16)
            nc.vector.memset(ones_sb[:], 1.0)
            for j in range(1, 4):
                nc.sync.dma_start(
                    xs_sb[:, j * 1024 : (j + 1) * 1024],
                    xs_d[:, j * 1024 : (j + 1) * 1024],
                )
            nc.sync.dma_start(xo_sb[:, 1024:2048], xo_d[:, 1024:2048])
            xr_sb = cp.tile([C, NL], F32)
            nc.sync.dma_start(xr_sb[:], xr_d[:])

            # q/k replicated at partition offsets 0/32/64/96 (score row
            # groups); vT blocks [128, 65] with trailing ones column.
            q_rep = cp.tile([96 + D, NL], BF16)
            k_rep = cp.tile([96 + D, N], BF16)
            vT = cp.tile([MB, N_MB * CA], E_DTYPE)

            # wk_sb/wq_sb hold 4 copies of the weights at col offsets
            # 0/32/64/96, so one matmul lands k/q at all four partition
            # groups and one CAST moves them to SBUF -- no replication DMAs.
            def prep_k_chunk(c):
                kp = pre_pool.tile([96 + D, NT], F32, tag="pre", name=f"kp{c}")
                nc.tensor.matmul(
                    kp[:], wk_sb[:], xs_sb[:, c * NT : (c + 1) * NT],
                    start=True, stop=True,
                )
                nc.vector.tensor_copy(k_rep[:, c * NT : (c + 1) * NT], kp[:])

            def prep_q_chunk(c):
                qp = pre_pool.tile([96 + D, NT], F32, tag="pre", name=f"qp{c}")
                nc.tensor.matmul(
                    qp[:], wq_sb[:], xo_sb[:, c * NT : (c + 1) * NT],
                    start=True, stop=True,
                )
                nc.vector.tensor_copy(q_rep[:, c * NT : (c + 1) * NT], qp[:])

            def prep_vt_block(mb):
                vp = pre_pool.tile([MB, CA], F32, tag="pre", name=f"vp{mb}")
                nc.tensor.matmul(
                    vp[:], xs_sb[:, mb * MB : (mb + 1) * MB], wv_sb[:],
                    start=True, stop=True,
                )
                nc.vector.tensor_copy(vT[:, mb * CA : (mb + 1) * CA], vp[:])

            prep_k_chunk(0)
            prep_q_chunk(0)

            for nt in range(N_NT):
                n0, n1 = nt * NT, (nt + 1) * NT
                avA = avA_pool.tile([CA, NT], F32, tag="avA", name=f"avA{nt}")
                avB = avB_pool.tile([CA, NT], F32, tag="avB", name=f"avB{nt}")

                def emit_av(e_t, w, avA=avA, avB=avB):
                    for j in range(2):
                        mb = 2 * w + j
                        nc.tensor.matmul(
                            avA[:],
                            vT[0:64, mb * CA : (mb + 1) * CA],
                            e_t[0:64, j * NT : (j + 1) * NT],
                            start=(mb == 0), stop=(mb == N_MB - 1),
                        )
                        nc.tensor.matmul(
                            avB[:],
                            vT[64:MB, mb * CA : (mb + 1) * CA],
                            e_t[64:MB, j * NT : (j + 1) * NT],
                            start=(mb == 0), stop=(mb == N_MB - 1),
                        )

                pend = []
                for p in range(N_MB // 4):  # wave pairs: m-blocks 4p..4p+3
                    if nt == 0:
                        if p + 1 < 8:
                            prep_k_chunk(p + 1)
                        if p in (1, 3, 5) and p // 2 + 1 < N_NT:
                            prep_q_chunk(p // 2 + 1)
                    # score quad: 4 concurrent rank-8 matmuls in distinct
                    # 32-row PE groups
                    st0 = st_pool.tile([MB, 2 * NT], F32, tag="st", name=f"st{nt}_{p}a")
                    st1 = st_pool.tile([MB, 2 * NT], F32, tag="st", name=f"st{nt}_{p}b")
                    for j4 in range(4):
                        mb = 4 * p + j4
                        rg = 32 * j4
                        stt, col = (st0, j4 * NT) if j4 < 2 else (st1, (j4 - 2) * NT)
                        nc.tensor.matmul(
                            stt[:, col : col + NT],
                            k_rep[rg : rg + D, mb * MB : (mb + 1) * MB],
                            q_rep[rg : rg + D, n0:n1],
                            start=True, stop=True,
                            tile_position=(rg, 0),
                        )
                    if nt == 0:
                        for mb in range(4 * p, 4 * p + 4):
                            prep_vt_block(mb)
                    e0 = e_pool.tile([MB, 2 * NT], E_DTYPE, tag="e", name=f"e{nt}_{p}a")
                    nc.scalar.activation(e0[:], st0[:], AF.Exp)
                    e1 = e_pool.tile([MB, 2 * NT], E_DTYPE, tag="e", name=f"e{nt}_{p}b")
                    nc.scalar.activation(e1[:], st1[:], AF.Exp)
                    for args in pend:
                        emit_av(*args)
                    pend = [(e0, 2 * p), (e1, 2 * p + 1)]
                for args in pend:
                    emit_av(*args)

                # normalize: out = unnorm/denom + x_opt  (gamma pre-folded)
                avAs = o_pool.tile([CA, NT], F32, tag="avAs", name=f"avAs{nt}")
                nc.vector.tensor_copy(avAs[:], avA[:])
                avS = o_pool.tile([CA, NT], F32, tag="avS", name=f"avS{nt}")
                nc.vector.tensor_add(avS[:], avB[:], avAs[:])
                recip = sm_pool.tile([1, NT], F32, tag="recip", name=f"recip{nt}")
                nc.vector.reciprocal(recip[:], avS[C:CA, :])
                recip_bf = sm_pool.tile([1, NT], BF16, tag="recip_bf", name=f"rb{nt}")
                nc.vector.tensor_copy(recip_bf[:], recip[:])
                bc = bc_pool.tile([C, NT], F32, tag="bc", name=f"bc{nt}")
                nc.tensor.matmul(bc[:], ones_sb[:], recip_bf[:], start=True, stop=True)
                bcs = o_pool.tile([C, NT], F32, tag="bcs", name=f"bcs{nt}")
                nc.vector.tensor_copy(bcs[:], bc[:])
                om = o_pool.tile([C, NT], F32, tag="om", name=f"om{nt}")
                nc.vector.tensor_mul(om[:], avS[0:C, :], bcs[:])
                o = o_pool.tile([C, NT], F32, tag="o", name=f"o{nt}")
                nc.vector.tensor_add(o[:], om[:], xr_sb[:, n0:n1])
                nc.sync.dma_start(out_d[:, n0:n1], o[:])
    nc.compile()
    return nc


_NC = None


def _get_nc() -> bass.Bass:
    global _NC
    if _NC is None:
        _NC = build_program()
    return _NC


def _to_bf16(a: np.ndarray) -> np.ndarray:
    """Round-to-nearest-even fp32 -> bf16 (ml_dtypes view)."""
    import ml_dtypes

    u = np.ascontiguousarray(a, np.float32).view(np.uint32)
    rounded = ((u + 0x7FFF + ((u >> 16) & 1)) >> 16).astype(np.uint16)
    return rounded.view(ml_dtypes.bfloat16)


def make_in_maps(x_opt, x_sar, wq, bq, wk, bk, wv, bv, gamma):
    f = np.float32
    x_opt = np.asarray(x_opt, f).reshape(B, C, N)
    x_sar = np.asarray(x_sar, f).reshape(B, C, N)
    g = float(np.asarray(gamma, f).reshape(()))
    wq_aug = np.concatenate([np.asarray(wq, f).T, np.asarray(bq, f)[None, :]], axis=0)
    wk_aug = np.concatenate([np.asarray(wk, f).T, np.asarray(bk, f)[None, :]], axis=0)
    # gamma folded into v (weights AND bias); denominator column stays 1.
    wv_aug = np.zeros((CA, CA), f)
    wv_aug[:C, :C] = np.asarray(wv, f).T * g
    wv_aug[C, :C] = np.asarray(bv, f) * g
    wv_aug[C, C] = 1.0
    wq4 = np.zeros((CA, 96 + D), f)
    wk4 = np.zeros((CA, 96 + D), f)
    for gidx in range(4):
        wq4[:, 32 * gidx : 32 * gidx + D] = wq_aug
        wk4[:, 32 * gidx : 32 * gidx + D] = wk_aug
    wq_bf = _to_bf16(wq4)
    wk_bf = _to_bf16(wk4)
    wv_bf = _to_bf16(wv_aug)
    ones_n = np.ones((1, N), f)
    maps = []
    for core in range(NCORES):
        b, h = divmod(core, 2)
        xo_aug = np.concatenate(
            [x_opt[b, :, h * NL : (h + 1) * NL], ones_n[:, :NL]], axis=0
        )
        xs_aug = np.concatenate([x_sar[b], ones_n], axis=0)
        maps.append(
            {
                "xo_bf": _to_bf16(xo_aug),
                "xs_bf": _to_bf16(xs_aug),
                "xores": np.ascontiguousarray(x_opt[b, :, h * NL : (h + 1) * NL]),
                "wq_bf": wq_bf,
                "wk_bf": wk_bf,
                "wv_bf": wv_bf,
            }
        )
    return maps


def assemble_out(results) -> np.ndarray:
    out = np.empty((B, C, N), np.float32)
    for core in range(NCORES):
        b, h = divmod(core, 2)
        out[b, :, h * NL : (h + 1) * NL] = results[core]["out"]
    return out.reshape(B, C, HH, WW)


def kernel(**inputs) -> np.ndarray:
    nc = _get_nc()
    maps = make_in_maps(**inputs)
    res = run_bass_kernel_spmd(nc, maps, list(range(NCORES)))
    return assemble_out(res.results)


# revision 10
# speedup vs baseline: 1.3254x; 1.0779x over previous
"""Trainium2 Bass kernel for CrossModalFusion (B=4, C=64, H=W=64, N=4096).

Reference computation (per sample b, with x reshaped to [C, N]):
    q = wq @ xo + bq          [8, N]
    k = wk @ xs + bk          [8, N]
    v = wv @ xs + bv          [64, N]
    S[n, m]  = q[:, n] . k[:, m]
    attn     = softmax_m(S)
    out      = gamma * (v @ attn^T) + x_opt

Sharding: 8 cores = 4 batch samples x 2 halves of the query (n) axis.
Each core computes output rows [64, 2048] for its (sample, n-half); no
cross-core communication is needed.

Per-core dataflow (matmuls in bf16 / f32r — the PE in this environment never
leaves the 1.2 GHz throttled clock, so concurrency via PE array tiling is the
main lever):
  - biases are folded into augmented weights on the host (ones-row trick);
    gamma is folded into wv/bv on the host, so the attention output comes out
    pre-scaled and the softmax denominator column stays unscaled.
  - scores are computed TRANSPOSED (S^T[m, n]) so the exp'd scores feed the
    attention*V matmuls directly as the moving operand.  v^T gets an extra
    ones column, so the AV matmuls' row 64 accumulate sum_m exp(S[n, m]) —
    the softmax denominator for free.  No max-subtraction: scores are O(3).
  - q/k are replicated at partition offsets 0/32/64/96 so four rank-8 S^T
    matmuls run concurrently in the four 32-row PE groups.
  - AV matmuls are split into rows 0-63 / 64-127 (two concurrent 64-row PE
    groups) accumulating into separate PSUM tiles avA/avB, summed at
    normalize time.
  - q/k/vT prep is interleaved just-in-time into n-tile 0's wave loop so the
    exp pipeline starts as soon as the first score block exists.
  - per n-tile of 512: accumulate over all 32 m-blocks, normalize by
    1/denominator, add the fp32 x_opt residual, DMA out.
"""

import os
import sys

import numpy as np

for _p in ("/opt/trn_rl_repo", "/root/.axon_site/_ro/trn_rl_repo"):
    if os.path.isdir(_p) and _p not in sys.path:
        sys.path.insert(0, _p)

import concourse.bass as bass
import concourse.mybir as mybir
import concourse.tile as tile
from concourse import bacc
from concourse.bass_utils import run_bass_kernel_spmd

F32 = mybir.dt.float32
F32R = mybir.dt.float32r
BF16 = mybir.dt.bfloat16
AF = mybir.ActivationFunctionType

B, C, HH, WW = 4, 64, 64, 64
N = HH * WW            # 4096 key/query positions
D = 8                  # q/k channel count
CA = C + 1             # augmented channel dim (ones row / denominator row)
NCORES = 8
NL = N // 2            # query rows per core
NT = 512               # n-tile (PSUM bank width in fp32)
MB = 128               # m-block (PE partition width)
N_NT = NL // NT        # 4 n-tiles per core
N_MB = N // MB         # 32 m-blocks
E_DTYPE = F32R         # exp output / AV operand dtype


def build_program(repeat: int = 1) -> bass.Bass:
    nc = bacc.Bacc("TRN2", target_bir_lowering=False, num_devices=NCORES)
    xo_d = nc.declare_dram_parameter("xo_bf", [CA, NL], BF16, isOutput=False)
    xs_d = nc.declare_dram_parameter("xs_bf", [CA, N], BF16, isOutput=False)
    xr_d = nc.declare_dram_parameter("xores", [C, NL], F32, isOutput=False)
    wq_d = nc.declare_dram_parameter("wq_bf", [CA, 96 + D], BF16, isOutput=False)
    wk_d = nc.declare_dram_parameter("wk_bf", [CA, 96 + D], BF16, isOutput=False)
    wv_d = nc.declare_dram_parameter("wv_bf", [CA, CA], BF16, isOutput=False)
    out_d = nc.declare_dram_parameter("out", [C, NL], F32, isOutput=True)

    with tile.TileContext(nc) as tc:
      for _rep in range(repeat):
        with (
            tc.tile_pool(name="const", bufs=1) as cp,
            tc.tile_pool(name="st_ps", bufs=2, space="PSUM") as st_pool,
            tc.tile_pool(name="avA_ps", bufs=1, space="PSUM") as avA_pool,
            tc.tile_pool(name="avB_ps", bufs=1, space="PSUM") as avB_pool,
            tc.tile_pool(name="bc_ps", bufs=1, space="PSUM") as bc_pool,
            tc.tile_pool(name="pre_ps", bufs=1, space="PSUM") as pre_pool,
            tc.tile_pool(name="e_sb", bufs=4) as e_pool,
            tc.tile_pool(name="o_sb", bufs=2) as o_pool,
            tc.tile_pool(name="sm_sb", bufs=2) as sm_pool,
        ):
            wk_sb = cp.tile([CA, 96 + D], BF16)
            nc.sync.dma_start(wk_sb[:], wk_d[:])
            wq_sb = cp.tile([CA, 96 + D], BF16)
            nc.sync.dma_start(wq_sb[:], wq_d[:])
            xs_sb = cp.tile([CA, N], BF16)
            xo_sb = cp.tile([CA, NL], BF16)
            nc.sync.dma_start(xs_sb[:, 0:1024], xs_d[:, 0:1024])
            nc.sync.dma_start(xo_sb[:, 0:1024], xo_d[:, 0:1024])
            wv_sb = cp.tile([CA, CA], BF16)
            nc.sync.dma_start(wv_sb[:], wv_d[:])
            ones_sb = cp.tile([1, C], BF16)
            nc.vector.memset(ones_sb[:], 1.0)
            for j in range(1, 4):
                nc.sync.dma_start(
                    xs_sb[:, j * 1024 : (j + 1) * 1024],
                    xs_d[:, j * 1024 : (j + 1) * 1024],
                )
            nc.sync.dma_start(xo_sb[:, 1024:2048], xo_d[:, 1024:2048])
            xr_sb = cp.tile([C, NL], F32)
            nc.sync.dma_start(xr_sb[:], xr_d[:])

            # q/k replicated at partition offsets 0/32/64/96 (score row
            # groups); vT blocks [128, 65] with trailing ones column.
            q_rep = cp.tile([96 + D, NL], BF16)
            k_rep = cp.tile([96 + D, N], BF16)
            vT = cp.tile([MB, N_MB * CA], E_DTYPE)

            # wk_sb/wq_sb hold 4 copies of the weights at col offsets
            # 0/32/64/96, so one matmul lands k/q at all four partition
            # groups and one CAST moves them to SBUF -- no replication DMAs.
            def prep_k_chunk(c, pool, tg):
                kp = pool.tile([96 + D, NT], F32, tag=tg, name=f"kp{c}")
                nc.tensor.matmul(
                    kp[:], wk_sb[:], xs_sb[:, c * NT : (c + 1) * NT],
                    start=True, stop=True,
                )
                nc.vector.tensor_copy(k_rep[:, c * NT : (c + 1) * NT], kp[:])

            def prep_q_chunk(c, pool, tg):
                qp = pool.tile([96 + D, NT], F32, tag=tg, name=f"qp{c}")
                nc.tensor.matmul(
                    qp[:], wq_sb[:], xo_sb[:, c * NT : (c + 1) * NT],
                    start=True, stop=True,
                )
                nc.vector.tensor_copy(q_rep[:, c * NT : (c + 1) * NT], qp[:])

            def prep_vt_block(mb, pool, tg):
                vp = pool.tile([MB, CA], F32, tag=tg, name=f"vp{mb}")
                nc.tensor.matmul(
                    vp[:], xs_sb[:, mb * MB : (mb + 1) * MB], wv_sb[:],
                    start=True, stop=True,
                )
                nc.vector.tensor_copy(vT[:, mb * CA : (mb + 1) * CA], vp[:])

            def prep_slot(i):
                # alternate between the pre bank and the (idle until tile
                # boundaries) bc bank so prep MMs pipeline with their casts
                return (pre_pool, "pre") if i % 2 == 0 else (bc_pool, "bc")

            _ps = [0]

            def next_slot():
                _ps[0] += 1
                return prep_slot(_ps[0])

            prep_k_chunk(0, *next_slot())
            prep_q_chunk(0, *next_slot())

            pending_norm = []
            norm_state = {}

            def norm_a(nt, avA, avB):
                # DVE-only half: sum the split accumulators, reciprocal
                avAs = o_pool.tile([CA, NT], F32, tag="avAs", name=f"avAs{nt}")
                nc.vector.tensor_copy(avAs[:], avA[:])
                avS = o_pool.tile([CA, NT], F32, tag="avS", name=f"avS{nt}")
                nc.vector.tensor_add(avS[:], avB[:], avAs[:])
                recip = sm_pool.tile([1, NT], F32, tag="recip", name=f"recip{nt}")
                nc.vector.reciprocal(recip[:], avS[C:CA, :])
                recip_bf = sm_pool.tile([1, NT], BF16, tag="recip_bf", name=f"rb{nt}")
                nc.vector.tensor_copy(recip_bf[:], recip[:])
                norm_state[nt] = (avS, recip_bf)

            def norm_b(nt, avA, avB):
                avS, recip_bf = norm_state.pop(nt)
                n0b, n1b = nt * NT, (nt + 1) * NT
                bc = bc_pool.tile([C, NT], F32, tag="bc", name=f"bc{nt}")
                nc.tensor.matmul(bc[:], ones_sb[:], recip_bf[:], start=True, stop=True)
                bcs = o_pool.tile([C, NT], F32, tag="bcs", name=f"bcs{nt}")
                nc.vector.tensor_copy(bcs[:], bc[:])
                om = o_pool.tile([C, NT], F32, tag="om", name=f"om{nt}")
                nc.vector.tensor_mul(om[:], avS[0:C, :], bcs[:])
                o = o_pool.tile([C, NT], F32, tag="o", name=f"o{nt}")
                nc.vector.tensor_add(o[:], om[:], xr_sb[:, n0b:n1b])
                nc.sync.dma_start(out_d[:, n0b:n1b], o[:])

            for nt in range(N_NT):
                n0, n1 = nt * NT, (nt + 1) * NT
                avA = avA_pool.tile([CA, NT], F32, tag="avA", name=f"avA{nt}")
                avB = avB_pool.tile([CA, NT], F32, tag="avB", name=f"avB{nt}")

                def emit_av(e_t, w, avA=avA, avB=avB):
                    for j in range(2):
                        mb = 2 * w + j
                        nc.tensor.matmul(
                            avA[:],
                            vT[0:64, mb * CA : (mb + 1) * CA],
                            e_t[0:64, j * NT : (j + 1) * NT],
                            start=(mb == 0), stop=(mb == N_MB - 1),
                        )
                        nc.tensor.matmul(
                            avB[:],
                            vT[64:MB, mb * CA : (mb + 1) * CA],
                            e_t[64:MB, j * NT : (j + 1) * NT],
                            start=(mb == 0), stop=(mb == N_MB - 1),
                        )

                pend = []
                for p in range(N_MB // 4):  # wave pairs: m-blocks 4p..4p+3
                    if nt == 0:
                        if p + 1 < 8:
                            prep_k_chunk(p + 1, *next_slot())
                        if p in (1, 3, 5) and p // 2 + 1 < N_NT:
                            prep_q_chunk(p // 2 + 1, *next_slot())
                    # score quad: 4 concurrent rank-8 matmuls in distinct
                    # 32-row PE groups
                    st0 = st_pool.tile([MB, 2 * NT], F32, tag="st", name=f"st{nt}_{p}a")
                    st1 = st_pool.tile([MB, 2 * NT], F32, tag="st", name=f"st{nt}_{p}b")
                    for j4 in range(4):
                        mb = 4 * p + j4
                        rg = 32 * j4
                        stt, col = (st0, j4 * NT) if j4 < 2 else (st1, (j4 - 2) * NT)
                        nc.tensor.matmul(
                            stt[:, col : col + NT],
                            k_rep[rg : rg + D, mb * MB : (mb + 1) * MB],
                            q_rep[rg : rg + D, n0:n1],
                            start=True, stop=True,
                            tile_position=(rg, 0),
                        )
                    if nt == 0:
                        for mb in range(4 * p, 4 * p + 4):
                            prep_vt_block(mb, *next_slot())
                    e0 = e_pool.tile([MB, 2 * NT], E_DTYPE, tag="e", name=f"e{nt}_{p}a")
                    nc.scalar.activation(e0[:], st0[:], AF.Exp)
                    e1 = e_pool.tile([MB, 2 * NT], E_DTYPE, tag="e", name=f"e{nt}_{p}b")
                    nc.scalar.activation(e1[:], st1[:], AF.Exp)
                    if pending_norm and p == 0:
                        norm_a(*pending_norm[0])
                    if pending_norm and p == 3:
                        norm_b(*pending_norm.pop(0))
                    for args in pend:
                        emit_av(*args)
                    pend = [(e0, 2 * p), (e1, 2 * p + 1)]
                for args in pend:
                    emit_av(*args)

                pending_norm.append((nt, avA, avB))
                if nt == N_NT - 1:
                    while pending_norm:
                        norm_a(*pending_norm[0])
                        norm_b(*pending_norm.pop(0))
    nc.compile()
    return nc


_NC = None


def _get_nc() -> bass.Bass:
    global _NC
    if _NC is None:
        _NC = build_program()
    return _NC


def _to_bf16(a: np.ndarray) -> np.ndarray:
    """Round-to-nearest-even fp32 -> bf16 (ml_dtypes view)."""
    import ml_dtypes

    u = np.ascontiguousarray(a, np.float32).view(np.uint32)
    rounded = ((u + 0x7FFF + ((u >> 16) & 1)) >> 16).astype(np.uint16)
    return rounded.view(ml_dtypes.bfloat16)


def make_in_maps(x_opt, x_sar, wq, bq, wk, bk, wv, bv, gamma):
    f = np.float32
    x_opt = np.asarray(x_opt, f).reshape(B, C, N)
    x_sar = np.asarray(x_sar, f).reshape(B, C, N)
    g = float(np.asarray(gamma, f).reshape(()))
    wq_aug = np.concatenate([np.asarray(wq, f).T, np.asarray(bq, f)[None, :]], axis=0)
    wk_aug = np.concatenate([np.asarray(wk, f).T, np.asarray(bk, f)[None, :]], axis=0)
    # gamma folded into v (weights AND bias); denominator column stays 1.
    wv_aug = np.zeros((CA, CA), f)
    wv_aug[:C, :C] = np.asarray(wv, f).T * g
    wv_aug[C, :C] = np.asarray(bv, f) * g
    wv_aug[C, C] = 1.0
    wq4 = np.zeros((CA, 96 + D), f)
    wk4 = np.zeros((CA, 96 + D), f)
    for gidx in range(4):
        wq4[:, 32 * gidx : 32 * gidx + D] = wq_aug
        wk4[:, 32 * gidx : 32 * gidx + D] = wk_aug
    wq_bf = _to_bf16(wq4)
    wk_bf = _to_bf16(wk4)
    wv_bf = _to_bf16(wv_aug)
    ones_n = np.ones((1, N), f)
    maps = []
    for core in range(NCORES):
        b, h = divmod(core, 2)
        xo_aug = np.concatenate(
            [x_opt[b, :, h * NL : (h + 1) * NL], ones_n[:, :NL]], axis=0
        )
        xs_aug = np.concatenate([x_sar[b], ones_n], axis=0)
        maps.append(
            {
                "xo_bf": _to_bf16(xo_aug),
                "xs_bf": _to_bf16(xs_aug),
                "xores": np.ascontiguousarray(x_opt[b, :, h * NL : (h + 1) * NL]),
                "wq_bf": wq_bf,
                "wk_bf": wk_bf,
                "wv_bf": wv_bf,
            }
        )
    return maps


def assemble_out(results) -> np.ndarray:
    out = np.empty((B, C, N), np.float32)
    for core in range(NCORES):
        b, h = divmod(core, 2)
        out[b, :, h * NL : (h + 1) * NL] = results[core]["out"]
    return out.reshape(B, C, HH, WW)


def kernel(**inputs) -> np.ndarray:
    nc = _get_nc()
    maps = make_in_maps(**inputs)
    res = run_bass_kernel_spmd(nc, maps, list(range(NCORES)))
    return assemble_out(res.results)


# revision 11
# speedup vs baseline: 1.4245x; 1.0748x over previous
"""Trainium2 Bass kernel for CrossModalFusion (B=4, C=64, H=W=64, N=4096).

Reference computation (per sample b, with x reshaped to [C, N]):
    q = wq @ xo + bq          [8, N]
    k = wk @ xs + bk          [8, N]
    v = wv @ xs + bv          [64, N]
    S[n, m]  = q[:, n] . k[:, m]
    attn     = softmax_m(S)
    out      = gamma * (v @ attn^T) + x_opt

Sharding: 8 cores = 4 batch samples x 2 halves of the query (n) axis.
Each core computes output rows [64, 2048] for its (sample, n-half); no
cross-core communication is needed.

Per-core dataflow (matmuls in bf16 / f32r — the PE in this environment never
leaves the 1.2 GHz throttled clock, so concurrency via PE array tiling is the
main lever):
  - biases are folded into augmented weights on the host (ones-row trick);
    gamma is folded into wv/bv on the host, so the attention output comes out
    pre-scaled and the softmax denominator column stays unscaled.
  - scores are computed TRANSPOSED (S^T[m, n]) so the exp'd scores feed the
    attention*V matmuls directly as the moving operand.  v^T gets an extra
    ones column, so the AV matmuls' row 64 accumulate sum_m exp(S[n, m]) —
    the softmax denominator for free.  No max-subtraction: scores are O(3).
  - q/k are replicated at partition offsets 0/32/64/96 so four rank-8 S^T
    matmuls run concurrently in the four 32-row PE groups.
  - AV matmuls are split into rows 0-63 / 64-127 (two concurrent 64-row PE
    groups) accumulating into separate PSUM tiles avA/avB, summed at
    normalize time.
  - q/k/vT prep is interleaved just-in-time into n-tile 0's wave loop so the
    exp pipeline starts as soon as the first score block exists.
  - per n-tile of 512: accumulate over all 32 m-blocks, normalize by
    1/denominator, add the fp32 x_opt residual, DMA out.
"""

import os
import sys

import numpy as np

for _p in ("/opt/trn_rl_repo", "/root/.axon_site/_ro/trn_rl_repo"):
    if os.path.isdir(_p) and _p not in sys.path:
        sys.path.insert(0, _p)

import concourse.bass as bass
import concourse.mybir as mybir
import concourse.tile as tile
from concourse import bacc
from concourse.bass_utils import run_bass_kernel_spmd

F32 = mybir.dt.float32
F32R = mybir.dt.float32r
BF16 = mybir.dt.bfloat16
AF = mybir.ActivationFunctionType

B, C, HH, WW = 4, 64, 64, 64
N = HH * WW            # 4096 key/query positions
D = 8                  # q/k channel count
CA = C + 1             # augmented channel dim (ones row / denominator row)
NCORES = 8
NL = N // 2            # query rows per core
NT = 512               # n-tile (PSUM bank width in fp32)
MB = 128               # m-block (PE partition width)
N_NT = NL // NT        # 4 n-tiles per core
N_MB = N // MB         # 32 m-blocks
E_DTYPE = F32R         # exp output / AV operand dtype


def build_program(repeat: int = 1) -> bass.Bass:
    nc = bacc.Bacc("TRN2", target_bir_lowering=False, num_devices=NCORES)
    xo_d = nc.declare_dram_parameter("xo_bf", [CA, NL], BF16, isOutput=False)
    xs_d = nc.declare_dram_parameter("xs_bf", [CA, N], BF16, isOutput=False)
    xr_d = nc.declare_dram_parameter("xores", [C, NL], F32, isOutput=False)
    wq_d = nc.declare_dram_parameter("wq_bf", [CA, 96 + D], BF16, isOutput=False)
    wk_d = nc.declare_dram_parameter("wk_bf", [CA, 96 + D], BF16, isOutput=False)
    wv_d = nc.declare_dram_parameter("wv_bf", [CA, CA], BF16, isOutput=False)
    out_d = nc.declare_dram_parameter("out", [C, NL], F32, isOutput=True)

    with tile.TileContext(nc) as tc:
      for _rep in range(repeat):
        with (
            tc.tile_pool(name="const", bufs=1) as cp,
            tc.tile_pool(name="st_ps", bufs=3, space="PSUM") as st_pool,
            tc.tile_pool(name="avA_ps", bufs=1, space="PSUM") as avA_pool,
            tc.tile_pool(name="avB_ps", bufs=1, space="PSUM") as avB_pool,
            tc.tile_pool(name="e_sb", bufs=4) as e_pool,
            tc.tile_pool(name="o_sb", bufs=2) as o_pool,
            tc.tile_pool(name="sm_sb", bufs=2) as sm_pool,
        ):
            wk_sb = cp.tile([CA, 96 + D], BF16)
            nc.sync.dma_start(wk_sb[:], wk_d[:])
            wq_sb = cp.tile([CA, 96 + D], BF16)
            nc.sync.dma_start(wq_sb[:], wq_d[:])
            xs_sb = cp.tile([CA, N], BF16)
            xo_sb = cp.tile([CA, NL], BF16)
            nc.sync.dma_start(xs_sb[:, 0:1024], xs_d[:, 0:1024])
            nc.sync.dma_start(xo_sb[:, 0:1024], xo_d[:, 0:1024])
            wv_sb = cp.tile([CA, CA], BF16)
            nc.sync.dma_start(wv_sb[:], wv_d[:])
            ones_sb = cp.tile([1, C], BF16)
            nc.vector.memset(ones_sb[:], 1.0)
            for j in range(1, 4):
                nc.sync.dma_start(
                    xs_sb[:, j * 1024 : (j + 1) * 1024],
                    xs_d[:, j * 1024 : (j + 1) * 1024],
                )
            nc.sync.dma_start(xo_sb[:, 1024:2048], xo_d[:, 1024:2048])
            xr_sb = cp.tile([C, NL], F32)
            nc.sync.dma_start(xr_sb[:], xr_d[:])

            # q/k replicated at partition offsets 0/32/64/96 (score row
            # groups); vT blocks [128, 65] with trailing ones column.
            q_rep = cp.tile([96 + D, NL], BF16)
            k_rep = cp.tile([96 + D, N], BF16)
            vT = cp.tile([MB, N_MB * CA], E_DTYPE)

            # wk_sb/wq_sb hold 4 copies of the weights at col offsets
            # 0/32/64/96, so one matmul lands k/q at all four partition
            # groups and one CAST moves them to SBUF -- no replication DMAs.
            def prep_k_chunk(c):
                kp = st_pool.tile([96 + D, NT], F32, tag="st", name=f"kp{c}")
                nc.tensor.matmul(
                    kp[:], wk_sb[:], xs_sb[:, c * NT : (c + 1) * NT],
                    start=True, stop=True,
                )
                nc.vector.tensor_copy(k_rep[:, c * NT : (c + 1) * NT], kp[:])

            def prep_q_chunk(c):
                qp = st_pool.tile([96 + D, NT], F32, tag="st", name=f"qp{c}")
                nc.tensor.matmul(
                    qp[:], wq_sb[:], xo_sb[:, c * NT : (c + 1) * NT],
                    start=True, stop=True,
                )
                nc.vector.tensor_copy(q_rep[:, c * NT : (c + 1) * NT], qp[:])

            def prep_vt_block(mb):
                vp = st_pool.tile([MB, CA], F32, tag="st", name=f"vp{mb}")
                nc.tensor.matmul(
                    vp[:], xs_sb[:, mb * MB : (mb + 1) * MB], wv_sb[:],
                    start=True, stop=True,
                )
                nc.vector.tensor_copy(vT[:, mb * CA : (mb + 1) * CA], vp[:])

            prep_k_chunk(0)
            prep_q_chunk(0)

            pending_norm = []
            norm_state = {}

            def norm_a(nt, avA, avB):
                # DVE-only half: sum the split accumulators, reciprocal
                avAs = o_pool.tile([CA, NT], F32, tag="avAs", name=f"avAs{nt}")
                nc.vector.tensor_copy(avAs[:], avA[:])
                avS = o_pool.tile([CA, NT], F32, tag="avS", name=f"avS{nt}")
                nc.vector.tensor_add(avS[:], avB[:], avAs[:])
                recip = sm_pool.tile([1, NT], F32, tag="recip", name=f"recip{nt}")
                nc.vector.reciprocal(recip[:], avS[C:CA, :])
                recip_bf = sm_pool.tile([1, NT], BF16, tag="recip_bf", name=f"rb{nt}")
                nc.vector.tensor_copy(recip_bf[:], recip[:])
                norm_state[nt] = (avS, recip_bf)

            def norm_b(nt, avA, avB):
                avS, recip_bf = norm_state.pop(nt)
                n0b, n1b = nt * NT, (nt + 1) * NT
                bc = st_pool.tile([C, NT], F32, tag="st", name=f"bc{nt}")
                nc.tensor.matmul(bc[:], ones_sb[:], recip_bf[:], start=True, stop=True)
                bcs = o_pool.tile([C, NT], F32, tag="bcs", name=f"bcs{nt}")
                nc.vector.tensor_copy(bcs[:], bc[:])
                om = o_pool.tile([C, NT], F32, tag="om", name=f"om{nt}")
                nc.vector.tensor_mul(om[:], avS[0:C, :], bcs[:])
                o = o_pool.tile([C, NT], F32, tag="o", name=f"o{nt}")
                nc.vector.tensor_add(o[:], om[:], xr_sb[:, n0b:n1b])
                nc.sync.dma_start(out_d[:, n0b:n1b], o[:])

            for nt in range(N_NT):
                n0, n1 = nt * NT, (nt + 1) * NT
                avA = avA_pool.tile([CA, NT], F32, tag="avA", name=f"avA{nt}")
                avB = avB_pool.tile([CA, NT], F32, tag="avB", name=f"avB{nt}")

                def emit_av(e_t, w, avA=avA, avB=avB):
                    for j in range(2):
                        mb = 2 * w + j
                        nc.tensor.matmul(
                            avA[:],
                            vT[0:64, mb * CA : (mb + 1) * CA],
                            e_t[0:64, j * NT : (j + 1) * NT],
                            start=(mb == 0), stop=(mb == N_MB - 1),
                        )
                        nc.tensor.matmul(
                            avB[:],
                            vT[64:MB, mb * CA : (mb + 1) * CA],
                            e_t[64:MB, j * NT : (j + 1) * NT],
                            start=(mb == 0), stop=(mb == N_MB - 1),
                        )

                pend = []
                for p in range(N_MB // 4):  # wave pairs: m-blocks 4p..4p+3
                    if nt == 0:
                        if p + 1 < 8:
                            prep_k_chunk(p + 1)
                        if p in (1, 3, 5) and p // 2 + 1 < N_NT:
                            prep_q_chunk(p // 2 + 1)
                    # score quad: 4 concurrent rank-8 matmuls in distinct
                    # 32-row PE groups
                    st0 = st_pool.tile([MB, 2 * NT], F32, tag="st", name=f"st{nt}_{p}a")
                    st1 = st_pool.tile([MB, 2 * NT], F32, tag="st", name=f"st{nt}_{p}b")
                    for j4 in range(4):
                        mb = 4 * p + j4
                        rg = 32 * j4
                        stt, col = (st0, j4 * NT) if j4 < 2 else (st1, (j4 - 2) * NT)
                        nc.tensor.matmul(
                            stt[:, col : col + NT],
                            k_rep[rg : rg + D, mb * MB : (mb + 1) * MB],
                            q_rep[rg : rg + D, n0:n1],
                            start=True, stop=True,
                            tile_position=(rg, 0),
                        )
                    if nt == 0:
                        for mb in range(4 * p, 4 * p + 4):
                            prep_vt_block(mb)
                    e0 = e_pool.tile([MB, 2 * NT], E_DTYPE, tag="e", name=f"e{nt}_{p}a")
                    nc.scalar.activation(e0[:], st0[:], AF.Exp)
                    e1 = e_pool.tile([MB, 2 * NT], E_DTYPE, tag="e", name=f"e{nt}_{p}b")
                    nc.scalar.activation(e1[:], st1[:], AF.Exp)
                    if pending_norm and p == 0:
                        norm_a(*pending_norm[0])
                    if pending_norm and p == 3:
                        norm_b(*pending_norm.pop(0))
                    for args in pend:
                        emit_av(*args)
                    pend = [(e0, 2 * p), (e1, 2 * p + 1)]
                for args in pend:
                    emit_av(*args)

                pending_norm.append((nt, avA, avB))
                if nt == N_NT - 1:
                    while pending_norm:
                        norm_a(*pending_norm[0])
                        norm_b(*pending_norm.pop(0))
    nc.compile()
    return nc


_NC = None


def _get_nc() -> bass.Bass:
    global _NC
    if _NC is None:
        _NC = build_program()
    return _NC


def _to_bf16(a: np.ndarray) -> np.ndarray:
    """Round-to-nearest-even fp32 -> bf16 (ml_dtypes view)."""
    import ml_dtypes

    u = np.ascontiguousarray(a, np.float32).view(np.uint32)
    rounded = ((u + 0x7FFF + ((u >> 16) & 1)) >> 16).astype(np.uint16)
    return rounded.view(ml_dtypes.bfloat16)


def make_in_maps(x_opt, x_sar, wq, bq, wk, bk, wv, bv, gamma):
    f = np.float32
    x_opt = np.asarray(x_opt, f).reshape(B, C, N)
    x_sar = np.asarray(x_sar, f).reshape(B, C, N)
    g = float(np.asarray(gamma, f).reshape(()))
    wq_aug = np.concatenate([np.asarray(wq, f).T, np.asarray(bq, f)[None, :]], axis=0)
    wk_aug = np.concatenate([np.asarray(wk, f).T, np.asarray(bk, f)[None, :]], axis=0)
    # gamma folded into v (weights AND bias); denominator column stays 1.
    wv_aug = np.zeros((CA, CA), f)
    wv_aug[:C, :C] = np.asarray(wv, f).T * g
    wv_aug[C, :C] = np.asarray(bv, f) * g
    wv_aug[C, C] = 1.0
    wq4 = np.zeros((CA, 96 + D), f)
    wk4 = np.zeros((CA, 96 + D), f)
    for gidx in range(4):
        wq4[:, 32 * gidx : 32 * gidx + D] = wq_aug
        wk4[:, 32 * gidx : 32 * gidx + D] = wk_aug
    wq_bf = _to_bf16(wq4)
    wk_bf = _to_bf16(wk4)
    wv_bf = _to_bf16(wv_aug)
    ones_n = np.ones((1, N), f)
    maps = []
    for core in range(NCORES):
        b, h = divmod(core, 2)
        xo_aug = np.concatenate(
            [x_opt[b, :, h * NL : (h + 1) * NL], ones_n[:, :NL]], axis=0
        )
        xs_aug = np.concatenate([x_sar[b], ones_n], axis=0)
        maps.append(
            {
                "xo_bf": _to_bf16(xo_aug),
                "xs_bf": _to_bf16(xs_aug),
                "xores": np.ascontiguousarray(x_opt[b, :, h * NL : (h + 1) * NL]),
                "wq_bf": wq_bf,
                "wk_bf": wk_bf,
                "wv_bf": wv_bf,
            }
        )
    return maps


def assemble_out(results) -> np.ndarray:
    out = np.empty((B, C, N), np.float32)
    for core in range(NCORES):
        b, h = divmod(core, 2)
        out[b, :, h * NL : (h + 1) * NL] = results[core]["out"]
    return out.reshape(B, C, HH, WW)


def kernel(**inputs) -> np.ndarray:
    nc = _get_nc()
    maps = make_in_maps(**inputs)
    res = run_bass_kernel_spmd(nc, maps, list(range(NCORES)))
    return assemble_out(res.results)
